# revision 29
# baseline (speedup 1.0000x reference)
"""Segment mean-pool (BERT lattice embedding) Trainium2 Bass kernel.

Full-input contract: kernel(hidden[64,512,768] f32, word_ids[64,512] i32,
num_tokens=400) -> [64,400,768] f32.

Strategy: data-parallel over batch across 8 NeuronCores (8 samples each).
Per sample b the ragged segment mean  out[t] = mean_{s: wid[s]==t} hidden[s]
is computed as a matmul on the PE array with the MEAN WEIGHTS folded into the
one-hot matrix:

    A[s, k]   = (wid'[s] == k) / count[wid[s]]      (k = compact word rank)
    psum[h,k] = sum_j hid[j-chunk, h].T @ A[j-chunk, k]
    out[h, k] = psum[h, k]                          (plain PSUM->SBUF copy)

Key layout/precision choices (each measured on HW):
  - fp16 end-to-end on the heavy tensors: halves HBM traffic; values are O(1)
    means of N(0,1) so rel-err stays ~4e-4 (gate 2e-2).
  - [h, k] output orientation: stationary operand = hid chunk [128s x 128h],
    moving = one-hot [128s x W].  Full partition utilization, no ragged tail.
  - COMPACT word axis: word_ids are sorted; only ~290 of 400 words per sample
    have pieces.  Host remaps each sample's words to ranks [0, n_b) and the
    device works at static width W = max_b n_b (rounded up).  Host scatters
    rows back (index-side work).  ~24% fewer output bytes + narrower psum
    evictions and one-hot builds.
  - WINDOWED matmuls: piece-chunk j only touches a narrow compact-rank band.
    Chunk 0 runs full-width (its start=True must initialize the whole PSUM
    bank - writes exact zeros where it has no pieces); chunks 1-3 stream only
    their windows (pure accumulates into written bytes).
  - ONE DMA ring (sync) for input prefetch AND outputs, in program order:
    the ring drains FIFO, so the input prefetch gets full HBM bandwidth and
    outputs stream right behind - total bytes are HBM-bound either way, so
    input-first is the optimal schedule.  om pool is deep enough (2G) that
    evictions never block on output drains.
  - HAM warm-up matmuls at t~8.5us (PE clock-gate sits at 1.2 GHz until
    ~3.4us of sustained activity); their sink eviction is emitted AFTER the
    main loop on ACT so it cannot delay the first one-hot builds.
"""

import numpy as np

B, S, H, T = 64, 512, 768, 400
N_CORES = 8
B_LOC = B // N_CORES  # samples per core
P = 128
J = S // P  # contraction chunks per sample
G = H // P  # output h-groups per sample
GB = G // 2  # h-groups per output DMA batch

_CACHED = {}


def _prep_meta(word_ids):
    """Host index-side preprocessing of the 128 KB word_ids tensor.

    Returns (W, windows, uniq, wid_compact):
      - uniq[s]: sorted present words of sample s (np.unique)
      - wid_compact[s, i]: rank of word_ids[s, i] within uniq[s]
      - W: static compact width = max_s len(uniq[s]) rounded up to 8
      - windows[j]: compact-rank window [lo, hi) of piece-chunk j, union over
        ALL samples (one program serves all cores); windows[0] = (0, W).
    """
    wid = np.asarray(word_ids, np.int64).reshape(B, S)
    uniq = []
    wid_c = np.empty_like(wid)
    for s in range(B):
        u, inv = np.unique(wid[s], return_inverse=True)
        uniq.append(u)
        wid_c[s] = inv
    W = int(np.ceil(max(len(u) for u in uniq) / 8) * 8)
    windows = [(0, W)]
    for j in range(1, J):
        lo = int(wid_c[:, j * P].min())
        hi = int(wid_c[:, j * P + P - 1].max()) + 1
        windows.append((lo, hi))
    return W, tuple(windows), uniq, wid_c


def build_program(W, windows):
    """Build + compile the single-core Bass program (same NEFF on all cores)."""
    import concourse.bass as bass  # noqa: F401
    import concourse.mybir as mybir
    import concourse.tile as tile
    from concourse import bacc

    nc = bacc.Bacc(
        "TRN2",
        target_bir_lowering=False,
        debug=False,
        enable_asserts=False,
        num_devices=N_CORES,
    )
    f32 = mybir.dt.float32
    f16 = mybir.dt.float16

    # hidden host-prearranged as [B_LOC, P, J, H] fp16:
    # hid_pjh[b, p, j, h] = hidden[b, 128j + p, h] -> the per-sample DMA is one
    # fully linear 786 KB transfer with 6 KB/partition contiguous runs.
    hidden_t = nc.dram_tensor(
        "hidden_pjh", [B_LOC, P, J, H], f16, kind="ExternalInput"
    ).ap()
    # wl_pbj[p, b, j, 0] = compact rank of piece 128j+p, [.., 1] = its mean
    # weight 1/count (fp32: tensor_scalar scalar operands must be fp32).
    # One merged tensor = one DMA = one ~1-2us HBM completion receipt.
    wl_t = nc.dram_tensor(
        "wl_pbj", [P, B_LOC, J, 2], f32, kind="ExternalInput"
    ).ap()
    # out[b, p, g, k] = pooled'[b, k, 128g+p] fp16; host scatters back.
    out_t = nc.dram_tensor("out", [B_LOC, P, G, W], f16, kind="ExternalOutput").ap()
    # Scratch sink for the HAM warm-up matmuls (keeps them from being DCE'd).
    warm_t = nc.dram_tensor("warm_out", [P, 16], f16, kind="ExternalOutput").ap()

    with tile.TileContext(nc) as tc:
        with tc.tile_pool(name="const", bufs=1) as const_pool, \
             tc.tile_pool(name="hidp", bufs=B_LOC) as hid_pool, \
             tc.tile_pool(name="aTp", bufs=4) as aT_pool, \
             tc.tile_pool(name="outp", bufs=B_LOC) as out_pool, \
             tc.tile_pool(name="psum", bufs=8, space="PSUM") as psum_pool:

            # One-hot-build operands: iota fp16 (16-bit DVE path; compact
            # ranks < 2048 are exact in fp16).
            iota_t = const_pool.tile([P, W], f16, name="iota_t")
            nc.gpsimd.iota(
                iota_t,
                pattern=[[1, W]],
                base=0,
                channel_multiplier=0,
                allow_small_or_imprecise_dtypes=True,
            )

            # Prefetch the whole input shard up front (fits in SBUF): 8 x
            # 786 KB back-to-back.  ALL DMAs - inputs first, then outputs as
            # they are produced - go on the ONE sync ring: it drains in issue
            # order, so the prefetch gets full HBM bandwidth (outputs would
            # otherwise round-robin at packet granularity and halve the input
            # rate, starving the matmul pipeline).
            # Tiny index tensor FIRST: in ring-FIFO order it transfers (and
            # pays its completion receipt) before the bulk prefetch, so the
            # one-hot builds can start at ~9us instead of ~12.5us.
            wl_sb = const_pool.tile([P, B_LOC, J, 2], f32, name="wl_sb")
            nc.sync.dma_start(out=wl_sb, in_=wl_t)

            hids = []
            for b in range(B_LOC):
                hid = hid_pool.tile([P, J, H], f16, name=f"hid{b}", tag="hid")
                nc.sync.dma_start(out=hid, in_=hidden_t[b])
                hids.append(hid)

            # HAM warm-up: the PE clock-gate defaults to 4/8 (1.2 GHz) and
            # only reaches 8/8 after ~3.4 us of sustained matmul activity.
            # Burn a few dummy matmuls while sample 0 is still in flight.
            wps = psum_pool.tile([P, W], f32, name="wps", tag="ps")
            for _ in range(4):
                nc.tensor.matmul(wps, iota_t[:, :P], iota_t, start=True, stop=True)

            for b in range(B_LOC):
                hid = hids[b]
                aT = aT_pool.tile([P, J, W], f16, name="aT", tag="aT")
                for j in range(J):
                    lo, hi = windows[j]
                    # aT[p, j, k] = (iota[k] == wid'[b, 128j+p]) * w[b, 128j+p]
                    # built only over the chunk's compact-rank window.
                    nc.vector.tensor_scalar(
                        aT[:, j, lo:hi],
                        iota_t[:, lo:hi],
                        wl_sb[:, b, j, 0:1],
                        wl_sb[:, b, j, 1:2],
                        op0=mybir.AluOpType.is_equal,
                        op1=mybir.AluOpType.mult,
                    )
                om = out_pool.tile([P, G, W], f16, name="om", tag="om")
                for g in range(G):
                    ps = psum_pool.tile([P, W], f32, name="ps", tag="ps")
                    for j in range(J):
                        lo, hi = windows[j]
                        # j=0 (full width) start=True clears has_written for
                        # the whole bank and writes zeros where it has no
                        # pieces; j>=1 accumulate inside their windows.
                        nc.tensor.matmul(
                            ps[:, lo:hi],
                            hid[:, j, g * P : (g + 1) * P],
                            aT[:, j, lo:hi],
                            start=(j == 0),
                            stop=(j == J - 1),
                        )
                    # Plain PSUM->SBUF eviction (mean already applied via
                    # w).  DVE also builds aT, so give ACT 2 of every 3.
                    if g % 3 == 0:
                        nc.vector.tensor_copy(om[:, g, :], ps)
                    else:
                        nc.scalar.copy(om[:, g, :], ps)
                # One output DMA per sample (fully linear both sides) on the
                # SAME sync ring, FIFO behind the input prefetch: fewer
                # ~0.7us HWDGE issue slots on the Sync sequencer.
                nc.sync.dma_start(out=out_t[b], in_=om)

                if b == 1:
                    # Warm-up sink: emitted here (not before the loop) so it
                    # cannot delay the first aT builds, and early enough to
                    # return wps's PSUM slot to the rotation.  x0 scale:
                    # iota.T @ iota values overflow fp16.
                    wsb = const_pool.tile([P, 16], f16, name="wsb")
                    nc.scalar.mul(wsb, wps[:, :16], 0.0)
                    nc.scalar.dma_start(out=warm_t, in_=wsb)

    nc.compile()
    return nc


def _prep_in_maps(hidden, wid_c, wpiece):
    hidden = np.ascontiguousarray(np.asarray(hidden), dtype=np.float32).reshape(B, S, H)
    in_maps = []
    for i in range(N_CORES):
        sl = slice(i * B_LOC, (i + 1) * B_LOC)
        # [B_LOC, S, H] -> [B_LOC, P, J, H] with s = 128j + p, cast fp16.
        hs = hidden[sl].reshape(B_LOC, J, P, H).transpose(0, 2, 1, 3)
        hs = np.ascontiguousarray(hs, dtype=np.float16)
        # [B_LOC, S] -> [P, B_LOC, J, 2] with rank/weight interleaved.
        wj = wid_c[sl].reshape(B_LOC, J, P).transpose(2, 0, 1).astype(np.float32)
        wp = wpiece[sl].reshape(B_LOC, J, P).transpose(2, 0, 1).astype(np.float32)
        wl = np.ascontiguousarray(np.stack([wj, wp], axis=-1))
        in_maps.append({"hidden_pjh": hs, "wl_pbj": wl})
    return in_maps


def _piece_weights(word_ids):
    """1/count[wid[s]] per piece, from the index tensor only."""
    wid = np.ascontiguousarray(np.asarray(word_ids), dtype=np.int64).reshape(B, S)
    counts = np.zeros((B, T), np.int64)
    np.add.at(counts, (np.repeat(np.arange(B), S), wid.reshape(-1)), 1)
    recip = (1.0 / np.maximum(counts, 1)).astype(np.float32)
    return np.take_along_axis(recip, wid, axis=1)  # [B, S]


def run(hidden, word_ids, trace=False, **trace_kwargs):
    from concourse import bass_utils

    W, windows, uniq, wid_c = _prep_meta(word_ids)
    key = (W, windows)
    if _CACHED.get("key") != key:
        _CACHED["nc"] = build_program(W, windows)
        _CACHED["key"] = key
    nc = _CACHED["nc"]
    in_maps = _prep_in_maps(hidden, wid_c, _piece_weights(word_ids))
    res = bass_utils.run_bass_kernel_spmd(
        nc, in_maps, core_ids=list(range(N_CORES)), trace=trace, **trace_kwargs
    )
    # [N_CORES x [B_LOC, P, G, W]] -> scatter compact ranks back to [B, T, H].
    dev = np.concatenate([np.asarray(res.results[i]["out"]) for i in range(N_CORES)])
    dev = dev.transpose(0, 3, 2, 1).reshape(B, W, H).astype(np.float32)  # [B, W, H]
    out = np.zeros((B, T, H), np.float32)
    for s in range(B):
        u = uniq[s]
        out[s, u, :] = dev[s, : len(u), :]
    return out, res


def kernel(hidden, word_ids, num_tokens=None, **_unused):
    out, _ = run(hidden, word_ids, trace=False)
    return out
